# revision 26
# baseline (speedup 1.0000x reference)
"""GTR phylogenetic likelihood (Felsenstein pruning) on 8 TRN2 NeuronCores.

Strategy: shard the 50000 sites across 8 cores (6250 each, padded to
6272 = 16 blocks x 392 sites). Data layout on device: [(node,state), site]
with 128-partition tiles of 32 nodes x 4 states. Per tree level, messages
are computed by f32r block-diagonal matmuls on the PE; the per-parent
product of the two child messages is an ACT copy (PSUM->SBUF of the left
messages) plus a DVE tensor_mul (right messages PSUM x left copy SBUF).
Node orderings per level are chosen so siblings split into the lower/upper
half of the partition range ("left block | right block"), making every
product a contiguous-partition op. Levels 4..1 pack 2/4/8/16 site-blocks
into the full 128 partitions (weights W (x) I_g) so DVE lanes stay full.
One Ln activation with accum_out reduces all 6272 site log-likelihoods per
core to 16 partial sums; the host adds the 8x16 partials.
"""

import numpy as np

NUM_TIPS = 256
NUM_SITES = 50000
K = 4
DEPTH = 8
N_NODES = 2 * NUM_TIPS - 1
N_EDGES = N_NODES - 1

N_CORES = 8
SITES_PER_CORE = NUM_SITES // N_CORES      # 6250
NBLK = 16
S = 392                                     # sites per block (16*392 = 6272)
PAD_SITES = NBLK * S                        # 6272
XW = 8 * S                                  # 3136 cols per block row

# weight column layout
OFF_W8 = 0
OFF_W7 = 1024
OFF_W6 = 1536
OFF_W5 = 1792
OFF_W4 = 1920
OFF_W3 = 2048
OFF_W2 = 2176
OFF_W1 = 2304
OFF_PI = 2432
NW = 2448


# ---------------------------------------------------------------- host math

def _softmax(x):
    e = np.exp(x - x.max())
    return e / e.sum()


def _softplus(x):
    return np.logaddexp(0.0, x)


def _expm(A):
    """Pade-13 scaling-and-squaring matrix exponential (float64)."""
    b = [64764752532480000., 32382376266240000., 7771770303897600.,
         1187353796428800., 129060195264000., 10559470521600.,
         670442572800., 33522128640., 1323241920., 40840800.,
         960960., 16380., 182., 1.]
    nrm = np.linalg.norm(A, 1)
    s = 0
    if nrm > 5.4:
        s = int(np.ceil(np.log2(nrm / 5.4)))
    A = A / (2.0 ** s)
    n = A.shape[0]
    I = np.eye(n)
    A2 = A @ A
    A4 = A2 @ A2
    A6 = A2 @ A4
    U = A @ (A6 @ (b[13] * A6 + b[11] * A4 + b[9] * A2)
             + b[7] * A6 + b[5] * A4 + b[3] * A2 + b[1] * I)
    V = (A6 @ (b[12] * A6 + b[10] * A4 + b[8] * A2)
         + b[6] * A6 + b[4] * A4 + b[2] * A2 + b[0] * I)
    R = np.linalg.solve(V - U, V + U)
    for _ in range(s):
        R = R @ R
    return R


def _gtr_P(edge_lengths, rates, pi_logits):
    """Per-edge transition matrices P_e = expm(Q t_e), float64."""
    pi = _softmax(pi_logits.astype(np.float64))
    r = _softplus(rates.astype(np.float64))
    Q = np.zeros((K, K))
    iu = np.triu_indices(K, 1)
    Q[iu] = r
    Q = Q + Q.T
    Q = Q * pi[None, :]
    Q = Q - np.diag(Q.sum(axis=1))
    mu = -np.sum(pi * np.diag(Q))
    Q = Q / mu
    P = np.empty((N_EDGES, K, K))
    for e in range(N_EDGES):
        P[e] = _expm(Q * float(edge_lengths[e]))
    return P, pi


def _orderings():
    """L[d-1] = node ordering for depth d, with the sibling-split property:
    L_d = [left children of L_{d-1} | right children of L_{d-1}]."""
    L = [np.array([1, 2], dtype=np.int64)]
    for _ in range(2, DEPTH + 1):
        prev = L[-1]
        L.append(np.concatenate([2 * prev + 1, 2 * prev + 2]))
    return L


def _build_weights(P, pi, L):
    W = np.zeros((128, NW), np.float64)
    for d, off in [(8, OFF_W8), (7, OFF_W7), (6, OFF_W6), (5, OFF_W5)]:
        Ld = L[d - 1]
        nt = len(Ld) // 32
        for t in range(nt):
            for c in range(32):
                e = Ld[32 * t + c] - 1
                W[4 * c:4 * c + 4,
                  off + 128 * t + 4 * c: off + 128 * t + 4 * c + 4] = P[e].T
    # cascade stages k=0..3 -> levels 4..1, site-groups g = 2^(k+1)
    for k, off in [(0, OFF_W4), (1, OFF_W3), (2, OFF_W2), (3, OFF_W1)]:
        dl = 4 - k
        g = 2 ** (k + 1)
        ns = 2 ** dl                   # nodes per group at this level
        Ld = L[dl - 1]
        rows_per_g = ns * 4
        np_half = ns // 2              # parents per group
        colpg = np_half * 4
        for G in range(g):
            for cp in range(np_half):
                eL = Ld[cp] - 1
                eR = Ld[np_half + cp] - 1
                rL = G * rows_per_g + 4 * cp
                rR = G * rows_per_g + 4 * (np_half + cp)
                cL = G * colpg + 4 * cp
                cR = 64 + G * colpg + 4 * cp
                W[rL:rL + 4, off + cL:off + cL + 4] = P[eL].T
                W[rR:rR + 4, off + cR:off + cR + 4] = P[eR].T
    for G in range(16):
        W[4 * G:4 * G + 4, OFF_PI + G] = pi
    import ml_dtypes
    return W.astype(ml_dtypes.bfloat16)


def _pack_tips(tip_partials, L):
    """-> per-core arrays [NBLK, 128, XW] bfloat16 (the kernel consumes
    tips at bf16 precision; casting host-side halves the HBM traffic)."""
    import ml_dtypes
    order = L[DEPTH - 1] - (NUM_TIPS - 1)
    X = tip_partials[order].transpose(0, 2, 1).reshape(4 * NUM_TIPS, NUM_SITES)
    X = np.ascontiguousarray(X).astype(ml_dtypes.bfloat16)
    cores = []
    for c in range(N_CORES):
        Xc = np.ones((4 * NUM_TIPS, PAD_SITES), ml_dtypes.bfloat16)
        Xc[:, :SITES_PER_CORE] = X[:, c * SITES_PER_CORE:(c + 1) * SITES_PER_CORE]
        t = Xc.reshape(8, 128, NBLK, S).transpose(2, 1, 0, 3)
        cores.append(np.ascontiguousarray(t.reshape(NBLK, 128, XW)))
    return cores


# ---------------------------------------------------------------- bass kernel

def _gen_bass():
    import concourse.bass as bass  # noqa: F401
    import concourse.tile as tile
    from concourse import bacc, mybir

    F32 = mybir.dt.float32
    BF16 = mybir.dt.bfloat16

    nc = bacc.Bacc("TRN2", target_bir_lowering=False, debug=False,
                   num_devices=N_CORES)
    tips = nc.dram_tensor("tips", [NBLK, 128, XW], BF16, kind="ExternalInput")
    wts = nc.dram_tensor("wts", [128, NW], BF16, kind="ExternalInput")
    out_acc = nc.dram_tensor("out_acc", [16, 1], F32, kind="ExternalOutput")

    with tile.TileContext(nc) as tc:
        from contextlib import ExitStack
        with ExitStack() as ctx:
            wpool = ctx.enter_context(tc.tile_pool(name="w", bufs=1))
            xpool = ctx.enter_context(tc.tile_pool(name="x", bufs=6))
            spool = ctx.enter_context(tc.tile_pool(name="s", bufs=4))
            cpool = ctx.enter_context(tc.tile_pool(name="c", bufs=6))
            ps2 = ctx.enter_context(tc.tile_pool(name="ps2", bufs=8,
                                                 space="PSUM"))
            ps = ps2

            wt = wpool.tile([128, NW], BF16)
            nc.sync.dma_start(wt[:], wts[:])

            def w8(j):
                return wt[:, OFF_W8 + 128 * j: OFF_W8 + 128 * (j + 1)]

            def w7(j):
                return wt[:, OFF_W7 + 128 * j: OFF_W7 + 128 * (j + 1)]

            def pair_atom1(wL, wR, rhsL, rhsR, out, ctag):
                """1-pair atom, separate 1-bank psum tiles: mL's bank frees
                after the ACT copy (its only reader), mR's after the DVE mul;
                the copy is emitted before MM_R so it overlaps it."""
                mL = ps2.tile([128, S], F32, tag="ps1", name="mLa")
                nc.tensor.matmul(mL[:], lhsT=wL, rhs=rhsL,
                                 start=True, stop=True)
                cL = cpool.tile([128, S], BF16, tag=ctag, name="cL")
                nc.scalar.copy(cL[:], mL[:])
                mR = ps2.tile([128, S], F32, tag="ps1", name="mRa")
                nc.tensor.matmul(mR[:], lhsT=wR, rhs=rhsR,
                                 start=True, stop=True)
                nc.vector.tensor_mul(out[:], mR[:], cL[:])

            # software pipeline state
            x_t = [None] * NBLK
            p7_t = [None] * NBLK
            p6_t = [None] * NBLK
            p5_t = [None] * NBLK
            casc = [None, None, None, None]   # p4, p3, p2, p1 current tiles
            rootbox = [None]

            def _cascade(b):
                for k in range(4):
                    period = 2 ** (k + 1)
                    if (b + 1) % period != 0:
                        break
                    off = [OFF_W4, OFF_W3, OFF_W2, OFF_W1][k]
                    mm = ps.tile([128, S], F32, tag="ps1", name="mmk")
                    nc.tensor.matmul(mm[:], lhsT=wt[:, off:off + 128],
                                     rhs=casc[k][:], start=True, stop=True)
                    cc = cpool.tile([64, S], BF16, tag=f"cc{k}")
                    nc.scalar.copy(cc[:], mm[0:64, :])
                    n_fire = (b + 1) // period - 1
                    if k < 3:
                        if n_fire % 2 == 0:
                            casc[k + 1] = spool.tile([128, S], BF16,
                                                     tag=f"p{3 - k}",
                                                     name=f"p{3 - k}")
                        h = (n_fire % 2) * 64
                        nc.vector.tensor_mul(casc[k + 1][h:h + 64, :],
                                             mm[64:128, :], cc[:])
                    else:
                        root = spool.tile([64, S], BF16, tag="root")
                        nc.vector.tensor_mul(root[:], mm[64:128, :], cc[:])
                        rootbox[0] = root

            for it in range(NBLK + 4):
                # ---- stage 0 @ it: DMA + level 8 (4 pair atoms)
                if it < NBLK:
                    b = it
                    x = xpool.tile([128, XW], BF16, tag="x")
                    nc.sync.dma_start(x[:], tips[b])
                    x_t[b] = x
                    t7 = spool.tile([128, 4, S], BF16, tag="p7")
                    p7_t[b] = t7
                    for j in range(4):
                        pair_atom1(w8(j), w8(j + 4),
                                   x[:, S * j:S * (j + 1)],
                                   x[:, S * (j + 4):S * (j + 5)],
                                   t7[:, j, :], "c8")
                    x_t[b] = None   # L8 is x's only consumer

                # ---- stage 1 @ it: level 7 of block it-1 (one 2-group atom)
                if 1 <= it <= NBLK:
                    b = it - 1
                    t7 = p7_t[b]
                    p6 = spool.tile([128, 2, S], BF16, tag="p6")
                    p6_t[b] = p6
                    for j in range(2):
                        pair_atom1(w7(j), w7(j + 2),
                                   t7[:, j, :], t7[:, j + 2, :],
                                   p6[:, j, :], "c7")
                    p7_t[b] = None

                # ---- stage 2 @ it: level 6 of block it-2 (1 pair atom)
                if 2 <= it <= NBLK + 1:
                    b = it - 2
                    p6 = p6_t[b]
                    p5 = spool.tile([128, S], BF16, tag="p5")
                    p5_t[b] = p5
                    pair_atom1(wt[:, OFF_W6:OFF_W6 + 128],
                               wt[:, OFF_W6 + 128:OFF_W6 + 256],
                               p6[:, 0, :], p6[:, 1, :], p5[:], "c6")
                    p6_t[b] = None

                # ---- stage 4 @ it: cascade firings after block it-4
                # (emitted before stage 3: L5 of block it-3 allocates the next
                #  casc[0]; the cascade of block it-4 must consume the old one)
                if 4 <= it <= NBLK + 3:
                    _cascade(it - 4)

                # ---- stage 3 @ it: level 5 of block it-3
                if 3 <= it <= NBLK + 2:
                    b = it - 3
                    m5 = ps.tile([128, S], F32, tag="ps1", name="m5")
                    nc.tensor.matmul(m5[:], lhsT=wt[:, OFF_W5:OFF_W5 + 128],
                                     rhs=p5_t[b][:], start=True, stop=True)
                    c5 = cpool.tile([64, S], BF16, tag="c5")
                    nc.scalar.copy(c5[:], m5[0:64, :])
                    if b % 2 == 0:
                        casc[0] = spool.tile([128, S], BF16, tag="p4",
                                             name="p4")
                    half = (b % 2) * 64
                    nc.vector.tensor_mul(casc[0][half:half + 64, :],
                                         m5[64:128, :], c5[:])
                    p5_t[b] = None

            # ---- site likelihoods + log + accumulate
            root = rootbox[0]
            mlik = ps.tile([16, S], F32, tag="ps1", name="mlik")
            nc.tensor.matmul(mlik[:], lhsT=wt[0:64, OFF_PI:OFF_PI + 16],
                             rhs=root[:], start=True, stop=True)
            lnout = spool.tile([16, S], F32, tag="lnout")
            acc = spool.tile([16, 1], F32, tag="acc")
            nc.scalar.activation(out=lnout[:], in_=mlik[:],
                                 func=mybir.ActivationFunctionType.Ln,
                                 accum_out=acc[:])
            nc.sync.dma_start(out_acc[:], acc[:])

    nc.compile()
    return nc


_BASS_CACHE = {}


def _gen_warmup():
    """Small SPMD kernel exercising PE/PSUM/ACT/DVE; a successful
    load/exec/unload clears wedged engine state left by a faulted
    execution (a DMA-only kernel does not)."""
    import concourse.tile as tile
    from concourse import bacc, mybir
    F32 = mybir.dt.float32
    BF16 = mybir.dt.bfloat16
    nc = bacc.Bacc("TRN2", target_bir_lowering=False, debug=False,
                   num_devices=1)
    a = nc.dram_tensor("wua", [128, 128], F32, kind="ExternalInput")
    b = nc.dram_tensor("wub", [128, 128], F32, kind="ExternalOutput")
    with tile.TileContext(nc) as tc:
        with tc.tile_pool(name="sb", bufs=1) as sb, \
             tc.tile_pool(name="pw", bufs=2, space="PSUM") as pw:
            t = sb.tile([128, 128], BF16)
            nc.gpsimd.dma_start(t[:], a[:])
            m = pw.tile([128, 128], F32)
            nc.tensor.matmul(m[:], lhsT=t[:], rhs=t[:], start=True, stop=True)
            c = sb.tile([128, 128], BF16)
            nc.scalar.copy(c[:], m[:])
            p = sb.tile([128, 128], F32)
            nc.vector.tensor_mul(p[:], m[:], c[:])
            nc.sync.dma_start(b[:], p[:])
    nc.compile()
    return nc


def _run_warmup():
    try:
        from concourse.bass_utils import run_bass_kernel_spmd
        if "wu" not in _BASS_CACHE:
            _BASS_CACHE["wu"] = _gen_warmup()
        a = np.ones((128, 128), np.float32)
        # single-core: a wedged core 0 makes any multi-core execute fail,
        # so the clearing run must itself be single-core (device 0)
        run_bass_kernel_spmd(_BASS_CACHE["wu"], [{"wua": a}], core_ids=[0])
        return True
    except Exception:
        return False


def _subprocess_exec(core_tips, W):
    """Last-resort: run in a fresh interpreter (clears any client-side
    poisoned jax/NRT state; NEFF comes from the on-disk cache)."""
    import os
    import subprocess
    import sys
    import tempfile
    d = tempfile.mkdtemp(prefix="gtrk_")
    inp = os.path.join(d, "in.npz")
    outp = os.path.join(d, "out.npz")
    np.savez(inp, W=W, **{f"t{i}": t for i, t in enumerate(core_tips)})
    kdir = os.path.dirname(os.path.abspath(__file__))
    code = (
        "import sys, numpy as np\n"
        f"sys.path.insert(0, {kdir!r})\n"
        "import kernel as KM\n"
        f"z = np.load({inp!r})\n"
        "core_tips = [z[f't{i}'] for i in range(KM.N_CORES)]\n"
        "KM._run_warmup()\n"
        "res = KM._run_on_device(core_tips, z['W'], trace=False,\n"
        "                        _allow_subproc=False)\n"
        "np.savez(" + repr(outp) + ", *[r['out_acc'] for r in res.results])\n"
    )
    for _ in range(3):
        p = subprocess.run([sys.executable, "-c", code], timeout=1800,
                           capture_output=True)
        if p.returncode == 0 and os.path.exists(outp):
            z = np.load(outp)
            return [{"out_acc": z[k]} for k in z.files]
    raise RuntimeError(
        "device execution failed repeatedly; last stderr:\n"
        + p.stderr.decode()[-2000:])


class _ListResults:
    def __init__(self, results):
        self.results = results
        self.exec_time_ns = None
        self.mean_exec_time_ns = None
        self.max_exec_time_core_id = None
        self.instructions_and_trace = None


def _run_on_device(core_tips, W, trace=False, _allow_subproc=True):
    from concourse.bass_utils import run_bass_kernel_spmd
    if "nc" not in _BASS_CACHE:
        _BASS_CACHE["nc"] = _gen_bass()
    nc = _BASS_CACHE["nc"]
    in_maps = [{"tips": t, "wts": W} for t in core_tips]
    last = None
    for attempt in range(3):
        try:
            return run_bass_kernel_spmd(nc, in_maps,
                                        core_ids=list(range(N_CORES)),
                                        trace=trace and attempt == 0)
        except Exception as e:
            last = e
            _run_warmup()
    if _allow_subproc:
        return _ListResults(_subprocess_exec(core_tips, W))
    raise last


def kernel(edge_lengths, tip_partials, rates, pi_logits, _trace=False,
           _want_res=False):
    edge_lengths = np.asarray(edge_lengths)
    tip_partials = np.asarray(tip_partials, dtype=np.float32)
    rates = np.asarray(rates)
    pi_logits = np.asarray(pi_logits)

    P, pi = _gtr_P(edge_lengths.astype(np.float64), rates, pi_logits)
    L = _orderings()
    W = _build_weights(P, pi, L)
    core_tips = _pack_tips(tip_partials, L)

    res = _run_on_device(core_tips, W, trace=_trace)
    total = sum(float(r["out_acc"].sum()) for r in res.results)
    out = np.array(total, dtype=np.float32)
    if _want_res:
        return out, res
    return out
